# revision 21
# baseline (speedup 1.0000x reference)
"""AttentionBlock kernel for Trainium2, data-parallel over batch on 8 cores.

Problem (hardcoded): x [16, 512, 32, 32] f32, GroupNorm(32 groups) ->
qkv 1x1 conv (8 heads, head dim 64) -> softmax attention -> proj 1x1
conv -> residual.  Each core handles 2 batches; no collectives.

v2 restructure vs baseline:
  - default matmul dtype bf16 (rel err 2.3e-3 measured, 8x under the
    2e-2 gate; halves SBUF stream bytes + LDWEIGHTS time).
  - attention emitted as one software-pipelined stream per (batch, j):
    per s-chunk i: co-streamed qk pair matmul -> per-chunk exp
    ([128,2,512], ~1us) -> av pair, with the logits PSUM (wps) double
    buffered so the PE never waits on the exp (the baseline lost
    ~2.1us per chunk-group to a PE<->ACT ping-pong on a single-buffered
    4-bank wps).
  - j (t-half) is outer to hp, so proj of half j can run while the
    other half's attention streams; proj/qkv/v matmuls of the other
    batch/half are interleaved into the attention stream as PE filler.
  - optional fp8 DoubleRow av matmul (ATTN_AV=dr): exp emitted as
    e5m2 with a folded shift exp(x/8 - 7.5) (logit col-max spread is
    ~20 nats, fits e5m2 normals), v^T as e4m3; 2 s-chunks contracted
    per PE pass.
  - batch 1's GroupNorm (DVE) overlaps batch 0's attention.
"""

import os
import numpy as np

import concourse.bass as bass
import concourse.tile as tile
from concourse import mybir, bacc
from concourse.bass import ds, ts

F32 = mybir.dt.float32
AF = mybir.ActivationFunctionType
ALU = mybir.AluOpType

# ---- problem constants (hardcoded per contract) ----
B = 16          # global batch
BPC = 2         # batches per core
NCORES = 8
C = 512         # channels
HW = 32
L = HW * HW     # 1024 sequence positions
H = 8           # heads
CH = C // H     # 64 head dim
NG = 32         # groups
GS = C // NG    # 16 channels per group
EPS = 1e-5
CC = C // 128   # 4 channel chunks of 128
OC = 2 * C // 128  # 8 output chunks for q+k
LOGIT_SCALE = 1.0 / (CH ** 0.5)  # 1/8, folded into the exp

_DT_NAMES = {"f32": F32, "f32r": mybir.dt.float32r, "bf16": mybir.dt.bfloat16}
MM_DT = _DT_NAMES[os.environ.get("ATTN_MM_DT", "bf16")]
W_DT = MM_DT if MM_DT == mybir.dt.bfloat16 else F32

# av matmul mode: "mm" = MM_DT matmuls, "dr" = fp8 DoubleRow (e5m2
# exp-weights x e4m3 v, 2 s-chunks per pass).
AV_MODE = os.environ.get("ATTN_AV", "mm")
EXP_SHIFT = 7.5 if AV_MODE == "dr" else 0.0
EW_DT = mybir.dt.float8e5 if AV_MODE == "dr" else MM_DT
VT_DT = mybir.dt.float8e4 if AV_MODE == "dr" else MM_DT
# r0 row-64 -> partition-0 hop: "dma" (known good) or "direct" gpsimd
R0_MODE = os.environ.get("ATTN_R0", "dma")
# v^T columns per head: av channels + denominator ones column; dr mode pads
# to 66 so every head's column block has an even byte offset/stride (the
# dual-fp8 LDWEIGHTS ISA check rejects odd ones).
VT_NC = CH + 2 if AV_MODE == "dr" else CH + 1


def _ld(ap):
    """View a DRAM fp32 AP as the matmul dtype for direct DMA (f32r only)."""
    if MM_DT == mybir.dt.float32r:
        return ap.bitcast(MM_DT)
    return ap


def build_nc():
    nc = bacc.Bacc(name="attn_block")

    x_d = nc.dram_tensor("x", (BPC, C, L), F32, kind="ExternalInput")
    qkwt_d = nc.dram_tensor("qk_wt", (C, 2 * C), W_DT, kind="ExternalInput")
    qkb_d = nc.dram_tensor("qk_b", (2 * C,), F32, kind="ExternalInput")
    vwt_d = nc.dram_tensor("v_wt", (C, C), W_DT, kind="ExternalInput")
    vb_d = nc.dram_tensor("v_b", (C,), F32, kind="ExternalInput")
    pwt_d = nc.dram_tensor("p_wt", (C, C), W_DT, kind="ExternalInput")
    pb_d = nc.dram_tensor("p_b", (C,), F32, kind="ExternalInput")
    nw_d = nc.dram_tensor("n_w", (C,), F32, kind="ExternalInput")
    nb_d = nc.dram_tensor("n_b", (C,), F32, kind="ExternalInput")
    gmat_d = nc.dram_tensor("g_mat", (128, 8), F32, kind="ExternalInput")
    gmatt_d = nc.dram_tensor("g_mat_t", (8, 128), F32, kind="ExternalInput")
    out_d = nc.dram_tensor("out", (BPC, C, L), F32, kind="ExternalOutput")

    with tile.TileContext(nc) as tc:
        with (
            tc.tile_pool(name="wpool", bufs=1) as wpool,
            tc.tile_pool(name="big", bufs=1) as big,
            tc.tile_pool(name="work", bufs=2) as work,
            tc.tile_pool(name="small", bufs=2) as small,
            tc.tile_pool(name="psum", bufs=1, space="PSUM") as psum,
        ):
            # ---------- input DMAs (x0 split across both HWDGE queues) ----------
            x_tiles = []
            for b in range(BPC):
                x_sb = big.tile([128, CC, L], F32, tag="x_sb", bufs=2, name=f"x_sb{b}")
                x_tiles.append(x_sb)
            # x0 in 8 sub-chunks round-robined over all three DMA paths so
            # the GroupNorm stats (per 512-column sub) start ASAP.
            x0_engs = [nc.sync, nc.scalar, nc.gpsimd]
            for cc in range(CC):
                for sub in range(2):
                    eng = x0_engs[(2 * cc + sub) % 3]
                    eng.dma_start(
                        x_tiles[0][:, cc, ds(sub * 512, 512)],
                        x_d[0].rearrange("(cc p) l -> p cc l", p=128)[
                            :, cc, ds(sub * 512, 512)
                        ],
                    )
            qk_b = wpool.tile([128, OC], F32)
            nc.sync.dma_start(qk_b, qkb_d[:].rearrange("(oc p) -> p oc", p=128))
            p_b = wpool.tile([128, CC], F32)
            nc.sync.dma_start(p_b, pb_d[:].rearrange("(cc p) -> p cc", p=128))
            n_w = wpool.tile([128, CC], F32)
            nc.sync.dma_start(n_w, nw_d[:].rearrange("(cc p) -> p cc", p=128))
            n_b = wpool.tile([128, CC], F32)
            nc.sync.dma_start(n_b, nb_d[:].rearrange("(cc p) -> p cc", p=128))
            g_mat = wpool.tile([128, 8], F32)
            nc.sync.dma_start(g_mat, gmat_d[:])
            g_mat_t = wpool.tile([8, 128], F32)
            nc.sync.dma_start(g_mat_t, gmatt_d[:])
            vb_rep = wpool.tile([128, C], F32)
            nc.sync.dma_start(
                vb_rep,
                bass.AP(tensor=vb_d, offset=0, ap=[[0, 128], [1, C]]),
            )
            ones_sb = wpool.tile([128, 8, H], VT_DT)
            nc.vector.memset(ones_sb, 1.0)
            exp_bias = None
            if EXP_SHIFT:
                exp_bias = wpool.tile([128, 1], F32)
                nc.vector.memset(exp_bias, -EXP_SHIFT)
            # qk weights land per head pair so qkproj(hp0) can start the
            # moment GroupNorm finishes; x1/p_wt go via gpsimd SWDGE queues
            # to keep both HWDGE queues clear for the startup critical path.
            qk_wt = wpool.tile([128, CC, 2 * C], MM_DT)
            for hp in range(4):
                nc.scalar.dma_start(
                    qk_wt[:, :, ds(hp * 256, 256)],
                    _ld(qkwt_d[:, ds(hp * 256, 256)]).rearrange(
                        "(cc p) o -> p cc o", p=128
                    ),
                )
            v_wt = wpool.tile([128, CC, C], MM_DT)
            nc.sync.dma_start(v_wt, _ld(vwt_d[:]).rearrange("(cc p) o -> p cc o", p=128))
            for cc in range(CC):
                nc.gpsimd.dma_start(
                    x_tiles[1][:, cc],
                    x_d[1].rearrange("(cc p) l -> p cc l", p=128)[:, cc],
                )
            p_wt = wpool.tile([128, CC, C], MM_DT)
            nc.gpsimd.dma_start(
                p_wt, _ld(pwt_d[:]).rearrange("(cc p) o -> p cc o", p=128)
            )

            hid_tiles = [None] * BPC
            vt_tiles = [None] * BPC
            aall_tiles = [None] * BPC
            qkp_tiles = [[None] * 4 for _ in range(BPC)]

            # ================= GroupNorm (DVE + 2 tiny PE matmuls) =============
            gn_stats_tiles = {}

            def gn_stat_ops(b):
                """Per-(cc, sub) bn_stats closures, usable as DVE fillers."""
                x_sb = x_tiles[b]
                stats = small.tile([128, CC, 2, 6], F32, tag="stats")
                gn_stats_tiles[b] = stats
                return [
                    (lambda cc=cc, sub=sub: nc.vector.bn_stats(
                        out=stats[:, cc, sub],
                        in_=x_sb[:, cc, ds(sub * 512, 512)],
                    ))
                    for cc in range(CC) for sub in range(2)
                ]

            def group_norm(b):
                x_sb = x_tiles[b]
                stats = gn_stats_tiles[b]
                mv = small.tile([128, CC, 2], F32, tag="mv")
                for cc in range(CC):
                    nc.vector.bn_aggr(out=mv[:, cc], in_=stats[:, cc])
                # var -> E[x^2] in place
                msq = small.tile([128, CC], F32, tag="msq")
                nc.vector.tensor_tensor(msq, mv[:, :, 0], mv[:, :, 0], ALU.mult)
                nc.vector.tensor_tensor(mv[:, :, 1], mv[:, :, 1], msq, ALU.add)
                gsum_ps = psum.tile([8, 8], F32, tag="mm", bufs=2)
                nc.tensor.matmul(
                    gsum_ps, g_mat, mv.rearrange("p a s -> p (a s)"),
                    start=True, stop=True,
                )
                gm2 = small.tile([8, CC, 2], F32, tag="gm2")
                nc.vector.tensor_scalar_mul(
                    gm2.rearrange("j a s -> j (a s)"), gsum_ps, 1.0 / GS
                )
                gsq = small.tile([8, CC], F32, tag="gsq")
                nc.vector.tensor_tensor(gsq, gm2[:, :, 0], gm2[:, :, 0], ALU.mult)
                varg = small.tile([8, CC], F32, tag="varg")
                nc.vector.tensor_tensor(varg, gm2[:, :, 1], gsq, ALU.subtract)
                nc.vector.tensor_scalar_add(varg, varg, EPS)
                # rsqrt: magic constant + 3 Newton iterations (DVE only)
                y = small.tile([8, CC], F32, tag="rsqy")
                yi = y.bitcast(mybir.dt.int32)
                nc.vector.tensor_scalar(
                    yi, varg.bitcast(mybir.dt.int32), 1, None,
                    op0=ALU.logical_shift_right,
                )
                nc.vector.tensor_scalar(
                    yi, yi, 0x5F3759DF, -1, op0=ALU.subtract, op1=ALU.mult
                )
                t1 = small.tile([8, CC], F32, tag="rsqt")
                for _ in range(3):
                    nc.vector.tensor_tensor(t1, y, y, ALU.mult)
                    nc.vector.tensor_tensor(t1, t1, varg, ALU.mult)
                    nc.vector.tensor_scalar(
                        t1, t1, -0.5, 1.5, op0=ALU.mult, op1=ALU.add
                    )
                    nc.vector.tensor_tensor(y, y, t1, ALU.mult)
                nc.vector.tensor_copy(out=gm2[:, :, 1], in_=y)
                cstat_ps = psum.tile([128, 8], F32, tag="mm", bufs=2)
                nc.tensor.matmul(
                    cstat_ps, g_mat_t, gm2.rearrange("j a s -> j (a s)"),
                    start=True, stop=True,
                )
                cstat = cstat_ps.rearrange("p (a s) -> p a s", s=2)
                s_ch = small.tile([128, CC], F32, tag="s_ch")
                nc.vector.tensor_tensor(s_ch, cstat[:, :, 1], n_w, ALU.mult)
                t_ch = small.tile([128, CC], F32, tag="t_ch")
                nc.vector.tensor_tensor(t_ch, cstat[:, :, 0], s_ch, ALU.mult)
                nc.vector.tensor_tensor(t_ch, n_b, t_ch, ALU.subtract)
                hid = big.tile([128, CC, L], MM_DT, tag="hid", bufs=2,
                               name=f"hid{b}")
                for cc in range(CC):
                    nc.vector.tensor_scalar(
                        hid[:, cc], x_sb[:, cc], s_ch[:, ds(cc, 1)],
                        t_ch[:, ds(cc, 1)], op0=ALU.mult, op1=ALU.add,
                    )
                hid_tiles[b] = hid

            # ================= qkv projections =================
            def qkproj_piece(b, hp, oi, lc):
                """One [128, 512] q-or-k output chunk for head pair hp."""
                hid = hid_tiles[b]
                qk_pair = qkp_tiles[b][hp]
                if qk_pair is None:
                    # bufs=8: all 8 (batch, hp) tiles concurrently live, so a
                    # filler's bias-add never WAR-waits on the other batch's
                    # attention reads (which sit later in the in-order queues).
                    qk_pair = work.tile([128, 2, L], MM_DT, tag="qk_pair",
                                        bufs=8, name=f"qkp{b}_{hp}")
                    qkp_tiles[b][hp] = qk_pair
                oc = 2 * hp + oi
                qkps = psum.tile([128, 512], F32, tag="mm", bufs=2)
                for cc in range(CC):
                    nc.tensor.matmul(
                        qkps,
                        qk_wt[:, cc, ds(oc * 128, 128)],
                        hid[:, cc, ds(lc * 512, 512)],
                        start=(cc == 0), stop=(cc == CC - 1),
                    )
                nc.vector.tensor_scalar(
                    qk_pair[:, oi, ds(lc * 512, 512)], qkps,
                    qk_b[:, ds(oc, 1)], None, op0=ALU.add,
                )

            def vproj_piece(b, lc):
                """v^T for one l-chunk: out[l, heads*(CH+1)] with ones col."""
                hid = hid_tiles[b]
                vt = vt_tiles[b]
                if vt is None:
                    vt = big.tile([128, 8, H, VT_NC], VT_DT, tag="vt", bufs=2,
                                  name=f"vt{b}")
                    nc.vector.tensor_copy(out=vt[:, :, :, CH], in_=ones_sb)
                    vt_tiles[b] = vt
                vps = psum.tile([128, C], F32, tag="mm", bufs=2)
                for cc in range(CC):
                    nc.tensor.matmul(
                        vps, hid[:, cc, ds(lc * 128, 128)], v_wt[:, cc],
                        start=(cc == 0), stop=(cc == CC - 1),
                    )
                nc.vector.tensor_tensor(
                    vt[:, lc, :, 0:CH],
                    vps.rearrange("p (h c) -> p h c", c=CH),
                    vb_rep.rearrange("p (h c) -> p h c", c=CH),
                    ALU.add,
                )

            def proj_piece(b, lc, oc4):
                """proj + bias + residual for one [128, 512] output chunk."""
                a_all = aall_tiles[b]
                x_sb = x_tiles[b]
                pps = psum.tile([128, 512], F32, tag="mm", bufs=2)
                for cc in range(CC):
                    nc.tensor.matmul(
                        pps, p_wt[:, cc, ds(oc4 * 128, 128)],
                        a_all[:, cc, ds(lc * 512, 512)],
                        start=(cc == 0), stop=(cc == CC - 1),
                    )
                o_sb = work.tile([128, 512], F32, tag="o_sb", bufs=3)
                nc.vector.scalar_tensor_tensor(
                    o_sb, pps, p_b[:, ds(oc4, 1)],
                    x_sb[:, oc4, ds(lc * 512, 512)],
                    op0=ALU.add, op1=ALU.add,
                )
                nc.sync.dma_start(
                    out_d[b].rearrange("(cc p) l -> p cc l", p=128)[
                        :, oc4, ds(lc * 512, 512)
                    ],
                    o_sb,
                )

            # ================= attention stream =================
            def attn_stream(b, j, fillers, dve_fillers=None):
                """Pipelined attention for t-half j: for each head pair hp and
                s-chunk i: qk pair matmul -> exp -> av, with av emission
                delayed one unit so the PE never waits on the ACT.  `fillers`
                is a list of closures, one popped per step, each emitting a
                small independent PE chunk (qkv/proj of other halves);
                `dve_fillers` likewise for DVE-only work (one per 2 steps)."""
                dve_fillers = dve_fillers or []
                a_all = aall_tiles[b]
                if a_all is None:
                    a_all = big.tile([128, CC, L], MM_DT, tag="a_all", bufs=2,
                                     name=f"a_all{b}")
                    aall_tiles[b] = a_all
                pend = []  # pending av units

                def emit_av(unit):
                    if AV_MODE == "dr":
                        hp, p, ew, av_ab = unit
                        for hh in range(2):
                            nc.tensor.matmul(
                                av_ab[hh],
                                vt_tiles[b][:, ds(2 * p, 2), 2 * hp + hh],
                                ew[:, :, hh, :],
                                start=(p == 0), stop=(p == 3),
                                perf_mode=mybir.MatmulPerfMode.DoubleRow,
                            )
                        if p == 3:
                            normalize(hp, av_ab)
                    else:
                        hp, i, ew, av_ab = unit
                        for hh in range(2):
                            nc.tensor.matmul(
                                av_ab[hh],
                                vt_tiles[b][:, i, 2 * hp + hh],
                                ew[:, hh, :],
                                start=(i == 0), stop=(i == 7),
                            )
                        if i == 7:
                            normalize(hp, av_ab)

                def normalize(hp, av_ab):
                    for hh, av_ps in enumerate(av_ab):
                        r_full = small.tile([65, 512], F32, tag="r_sb", bufs=4)
                        nc.vector.reciprocal_approx_fast(r_full, av_ps[0:65])
                        if R0_MODE == "dma":
                            r0 = small.tile([1, 512], F32, tag="r0", bufs=4)
                            nc.sync.dma_start(r0, r_full[64:65])
                        else:
                            r0 = r_full[64:65]
                        r_rep = small.tile([64, 512], F32, tag="r_rep", bufs=4)
                        nc.gpsimd.partition_broadcast(r_rep, r0)
                        nc.vector.tensor_tensor(
                            a_all[ds(hh * 64, 64), hp, ds(j * 512, 512)],
                            av_ps[0:64], r_rep, ALU.mult,
                        )

                for hp in range(4):
                    qk_pair = qkp_tiles[b][hp]
                    av_ab = [
                        psum.tile([VT_NC, 512], F32, tag=t, bufs=1,
                                  name=f"av{t}{b}{j}{hp}")
                        for t in ("ava", "avb")
                    ]
                    ew_pair = None
                    for i in range(8):
                        if fillers:
                            f = fillers.pop(0)
                            if f is not None:
                                f()
                        if dve_fillers and i % 2 == 0:
                            dve_fillers.pop(0)()
                        # co-streamed qk pair (row-quadrant packed)
                        wps = psum.tile([128, 2, 512], F32, tag="wab", bufs=2)
                        for hh in range(2):
                            nc.tensor.matmul(
                                wps[:, hh],
                                qk_pair[ds(hh * 64, 64), 1, ds(i * 128, 128)],
                                qk_pair[ds(hh * 64, 64), 0, ds(j * 512, 512)],
                                start=True, stop=True,
                                tile_position=(hh * 64, 0),
                            )
                        if AV_MODE == "dr":
                            if i % 2 == 0:
                                ew_pair = work.tile([128, 2, 2, 512], EW_DT,
                                                    tag="ew", bufs=4)
                            nc.scalar.activation(
                                out=ew_pair[:, i % 2].rearrange("p a b -> p (a b)"),
                                in_=wps.rearrange("p a b -> p (a b)"),
                                func=AF.Exp, scale=LOGIT_SCALE, bias=exp_bias[:],
                            )
                            if i % 2 == 1:
                                pend.append((hp, i // 2, ew_pair, av_ab))
                        else:
                            ew = work.tile([128, 2, 512], EW_DT, tag="ew", bufs=4)
                            nc.scalar.activation(
                                out=ew.rearrange("p a b -> p (a b)"),
                                in_=wps.rearrange("p a b -> p (a b)"),
                                func=AF.Exp, scale=LOGIT_SCALE,
                            )
                            pend.append((hp, i, ew, av_ab))
                        if len(pend) > 1:
                            emit_av(pend.pop(0))
                while pend:
                    emit_av(pend.pop(0))
                while fillers:
                    f = fillers.pop(0)
                    if f is not None:
                        f()

            # ================= schedule =================
            def qkp(b, hp, oi, lc):
                return lambda: qkproj_piece(b, hp, oi, lc)

            def vp(b, lc):
                return lambda: vproj_piece(b, lc)

            def pp(b, lc, oc4):
                return lambda: proj_piece(b, lc, oc4)

            for op in gn_stat_ops(0):
                op()
            group_norm(0)
            # minimal prologue: only what attn(0,0) step 0 needs.  The q
            # projections for the j=1 half (oi=0, lc=1) are deferred to the
            # j=1 stream's fillers.
            qkproj_piece(0, 0, 0, 0)
            qkproj_piece(0, 0, 1, 0)
            qkproj_piece(0, 0, 1, 1)
            vproj_piece(0, 0)
            # fillers with deadlines: vt l-chunk i before the av of s-chunk
            # i; head pair hp's (q j=0, k both halves) before step 8*hp.
            fill = [
                vp(0, 1), vp(0, 2), vp(0, 3), qkp(0, 1, 0, 0),
                vp(0, 4), qkp(0, 1, 1, 0), vp(0, 5), qkp(0, 1, 1, 1),
                vp(0, 6), vp(0, 7),
                qkp(0, 2, 0, 0), qkp(0, 2, 1, 0), qkp(0, 2, 1, 1),
                qkp(0, 3, 0, 0), qkp(0, 3, 1, 0), qkp(0, 3, 1, 1),
            ]
            # batch 1's bn_stats drip through attn(0,0) as DVE fillers so the
            # GroupNorm(1) PE matmuls don't block the queue at the j-boundary
            attn_stream(0, 0, fill, dve_fillers=gn_stat_ops(1))
            group_norm(1)  # rest of GroupNorm(1); stats already done

            # j=1 q projections first (hp0's needed immediately), then all of
            # batch 1's qkv/v; a couple of empty slots let GroupNorm(1) land.
            fill = [qkp(0, 0, 0, 1), qkp(0, 1, 0, 1), qkp(0, 2, 0, 1),
                    qkp(0, 3, 0, 1), None, None]
            for hp in range(4):
                fill += [qkp(1, hp, 0, 0), qkp(1, hp, 1, 0),
                         qkp(1, hp, 1, 1), qkp(1, hp, 0, 1)]
                fill += [vp(1, 2 * hp), vp(1, 2 * hp + 1)]
            attn_stream(0, 1, fill)

            # proj(b0) as filler inside batch 1's first half; the lc=1 pieces
            # wait until attn(0, 1)'s last normalizes have surely landed.
            fill = [pp(0, 0, oc4) for oc4 in range(CC)]
            fill += [None] * 2
            fill += [pp(0, 1, oc4) for oc4 in range(CC)]
            attn_stream(1, 0, fill)

            # last stream: proj(1,1) for oc4 0/1 accumulates per-cc during
            # the stream as each head pair's a_all lands, so only the cc=3
            # matmul + epilogue remain after the last exp (shorter tail).
            pps_tail = {}

            def tail_mm(oc4, cc):
                def f():
                    if cc == 0:
                        pps_tail[oc4] = psum.tile([128, 512], F32, tag="mm",
                                                  bufs=2,
                                                  name=f"pps_tail{oc4}")
                    nc.tensor.matmul(
                        pps_tail[oc4], p_wt[:, cc, ds(oc4 * 128, 128)],
                        aall_tiles[1][:, cc, ds(512, 512)],
                        start=(cc == 0), stop=(cc == CC - 1),
                    )
                return f

            fill = [None] * 2
            fill += [pp(1, 0, oc4) for oc4 in range(CC)]
            fill += [None] * 14
            fill += [tail_mm(0, 0), tail_mm(0, 1), tail_mm(1, 0),
                     tail_mm(1, 1), None, None, tail_mm(0, 2), tail_mm(1, 2)]
            attn_stream(1, 1, fill)
            for oc4 in range(2):
                tail_mm(oc4, 3)()
                o_sb = work.tile([128, 512], F32, tag="o_sb", bufs=3)
                nc.vector.scalar_tensor_tensor(
                    o_sb, pps_tail[oc4], p_b[:, ds(oc4, 1)],
                    x_tiles[1][:, oc4, ds(512, 512)],
                    op0=ALU.add, op1=ALU.add,
                )
                nc.sync.dma_start(
                    out_d[1].rearrange("(cc p) l -> p cc l", p=128)[
                        :, oc4, ds(512, 512)
                    ],
                    o_sb,
                )
            for oc4 in range(2, CC):
                proj_piece(1, 1, oc4)

    nc.finalize()
    return nc


def prep_inputs(inputs):
    """Host-side weight permutation / transposition; returns per-core in_maps."""
    import numpy as np
    x = np.asarray(inputs["x"], np.float32).reshape(B, C, L)
    qkv_w = np.asarray(inputs["qkv_w"], np.float32)
    qkv_b = np.asarray(inputs["qkv_b"], np.float32)
    proj_w = np.asarray(inputs["proj_w"], np.float32)
    proj_b = np.asarray(inputs["proj_b"], np.float32)
    norm_w = np.asarray(inputs["norm_w"], np.float32)
    norm_b = np.asarray(inputs["norm_b"], np.float32)

    w3 = qkv_w.reshape(H, 3, CH, C)   # [head, (q,k,v), ch, c_in]
    b3 = qkv_b.reshape(H, 3, CH)
    # qk: per head pair -> [q_h0, q_h1, k_h0, k_h1] blocks of 64 rows
    qk_rows = []
    qk_brows = []
    for hp in range(4):
        for which in (0, 1):
            for h in (2 * hp, 2 * hp + 1):
                qk_rows.append(w3[h, which])
                qk_brows.append(b3[h, which])
    qk_w_perm = np.concatenate(qk_rows, 0)          # [1024, 512]
    qk_wt = np.ascontiguousarray(qk_w_perm.T)       # [512, 1024]
    qk_b = np.concatenate(qk_brows, 0)              # [1024]
    v_w_perm = w3[:, 2].reshape(C, C)               # head-major v rows
    v_wt = np.ascontiguousarray(v_w_perm.T)         # [512, 512]
    v_b = b3[:, 2].reshape(C)
    p_wt = np.ascontiguousarray(proj_w.T)
    g_mat = np.zeros((128, 8), np.float32)
    g_mat[np.arange(128), np.arange(128) // 16] = 1.0
    g_mat_t = np.ascontiguousarray(g_mat.T)

    if MM_DT == mybir.dt.bfloat16:
        import ml_dtypes
        bf = ml_dtypes.bfloat16
        qk_wt = qk_wt.astype(bf)
        v_wt = v_wt.astype(bf)
        p_wt = p_wt.astype(bf)
    shared = {
        "qk_wt": qk_wt, "qk_b": qk_b, "v_wt": v_wt, "v_b": v_b,
        "p_wt": p_wt, "p_b": proj_b, "n_w": norm_w, "n_b": norm_b,
        "g_mat": g_mat, "g_mat_t": g_mat_t,
    }
    in_maps = []
    for c in range(NCORES):
        m = dict(shared)
        m["x"] = np.ascontiguousarray(x[c * BPC: (c + 1) * BPC])
        in_maps.append(m)
    return in_maps


_NC_CACHE = {}


def get_nc():
    key = f"{MM_DT}-{AV_MODE}-{R0_MODE}"
    if key not in _NC_CACHE:
        _NC_CACHE[key] = build_nc()
    return _NC_CACHE[key]


def kernel(**inputs) -> np.ndarray:
    from concourse import bass_utils

    nc = get_nc()
    in_maps = prep_inputs(inputs)
    res = bass_utils.run_bass_kernel_spmd(nc, in_maps, core_ids=list(range(NCORES)))
    outs = [res.results[c]["out"] for c in range(NCORES)]
    full = np.concatenate(outs, 0).reshape(B, C, HW, HW)
    return full.astype(np.float32)
